# revision 17
# baseline (speedup 1.0000x reference)
"""Trainium2 Bass kernel for a diagonal SSM layer.

Reference computation (per batch row b, seq t):
    a_t = sigmoid(Wa @ x_t + bias)        [state=256]
    b_t = B @ x_t                         [state=256]
    h_t = a_t * h_{t-1} + b_t             (linear scan over t)
    y_t = C @ h_t + D @ x_t               [d_model=1024]

Distribution: data-parallel over batch (8 rows -> 8 NeuronCores),
weights replicated. Host pre-transposes and pre-quantizes the streams:
 - a-matmul runs fully in fp8(e4m3) DoubleRow mode (2 k-slabs per
   instruction, 2x the bf16 MAC rate); the sigmoid's flat slope at
   logit ~2.2 makes the quantization error negligible.
 - D-matmul contracts 6 of its 8 K-slabs in fp8 DoubleRow and 2 in
   bf16 -- the error budget (rel 2e-2 vs the f32 reference) allows fp8
   on only part of the dominant D@x term. The slab choice and the
   exponent shift were picked by an exact numpy simulation of the
   quantization error (which matches HW to ~4 digits).
 - b-matmul and C-matmul stay bf16; b's error is amplified ~1.3x by the
   scan so it cannot afford fp8.
 - fp8 operands are exponent-shifted (x*2^-2, weights*2^2) so the
   uniform-distributed weights clear the e4m3 subnormal cutoff.
The scan runs along the SBUF free dimension via the hardware
TensorTensorScan instruction. The PE pays ~190ns per bf16<->fp8 mode
transition, so DoubleRow matmuls are batched into long runs.

All HBM-side tensors are pre-arranged on the host into the exact
[partition, free] order the SBUF tiles use, so every DMA moves long
contiguous runs (2-8KB per partition) instead of 256-512B strided
rows -- the baseline's rearranging DMAs ran at ~73GB/s and starved
the PE for the first 12us.
"""

import sys
import types

sys.path.insert(0, "/opt/trn_rl_repo")


def _ensure_axon_hooks_shim():
    # Some images lack antenv.axon_hooks; concourse imports it
    # unconditionally when BASS_TRACE is set. Provide a no-op shim so
    # tracing degrades gracefully instead of crashing.
    try:
        import antenv.axon_hooks  # noqa: F401
        return
    except ImportError:
        pass
    import antenv

    mod = types.ModuleType("antenv.axon_hooks")
    mod._hook = None

    def get_axon_ntff_profile_hook():
        return mod._hook

    def set_axon_ntff_profile_hook(hook):
        mod._hook = hook

    mod.get_axon_ntff_profile_hook = get_axon_ntff_profile_hook
    mod.set_axon_ntff_profile_hook = set_axon_ntff_profile_hook
    sys.modules["antenv.axon_hooks"] = mod
    antenv.axon_hooks = mod


_ensure_axon_hooks_shim()

from contextlib import ExitStack

import numpy as np

from concourse import bacc, bass, mybir, tile
from concourse.bass_utils import run_bass_kernel_spmd

D_MODEL = 1024
STATE = 256
SEQ = 4096
BATCH = 8
N_CORES = 8
P = 128

KD = D_MODEL // P  # 8 k-slabs over d_model
KS = STATE // P  # 2 slabs over state
K8 = 6  # k-slabs of the D contraction done in fp8 DoubleRow (must be even)
KDB = KD - K8  # bf16 k-slabs for D
K8OFF = 2  # first fp8 k-slab (slabs K8OFF..K8OFF+K8-1 are fp8, rest bf16)
SD = 2  # fp8 exponent shift: x*2^-SD, weights*2^SD
CHUNKS = [256, 256] + [512] * 6 + [384, 128]
STARTS = [sum(CHUNKS[:i]) for i in range(len(CHUNKS))]
NCH = len(CHUNKS)

f32 = mybir.dt.float32
bf16 = mybir.dt.bfloat16
fp8 = mybir.dt.float8e4
ts = bass.ts
AF = mybir.ActivationFunctionType
ALU = mybir.AluOpType
DR = mybir.MatmulPerfMode.DoubleRow


def _build_nc():
    nc = bacc.Bacc("TRN2", target_bir_lowering=False, debug=False)

    # All inputs are host-pre-arranged [128, free] with the free dim in
    # the exact k-major order the SBUF tiles use: column c of row p is
    # contiguous, so each DMA is 128 descriptors of 2-8KB.
    x8 = nc.dram_tensor("x8", [P, KD * SEQ], fp8, kind="ExternalInput").ap()
    xbf = nc.dram_tensor("xbf", [P, KD * SEQ], bf16, kind="ExternalInput").ap()
    waT8 = nc.dram_tensor("waT8", [P, KD * STATE], fp8, kind="ExternalInput").ap()
    bT = nc.dram_tensor("bT", [P, KD * STATE], bf16, kind="ExternalInput").ap()
    cT = nc.dram_tensor("cT", [P, KS * D_MODEL], bf16, kind="ExternalInput").ap()
    dT8 = nc.dram_tensor("dT8", [P, K8 * D_MODEL], fp8, kind="ExternalInput").ap()
    dTbf = nc.dram_tensor("dTbf", [P, KDB * D_MODEL], bf16, kind="ExternalInput").ap()
    bias = nc.dram_tensor("bias", [P, KS], f32, kind="ExternalInput").ap()
    y = nc.dram_tensor("y", [SEQ, D_MODEL], f32, kind="ExternalOutput").ap()

    with tile.TileContext(nc) as tc, ExitStack() as ctx:
        wpool = ctx.enter_context(tc.tile_pool(name="w", bufs=1))
        xpool = ctx.enter_context(tc.tile_pool(name="x", bufs=4))
        apool = ctx.enter_context(tc.tile_pool(name="a", bufs=2))
        hpool = ctx.enter_context(tc.tile_pool(name="h", bufs=2))
        ypool = ctx.enter_context(tc.tile_pool(name="yo", bufs=2))
        hbfpool = ctx.enter_context(tc.tile_pool(name="hbf", bufs=2))
        pa = ctx.enter_context(tc.tile_pool(name="pa", bufs=1, space="PSUM"))
        pb = ctx.enter_context(tc.tile_pool(name="pb", bufs=1, space="PSUM"))
        py = ctx.enter_context(tc.tile_pool(name="py", bufs=4, space="PSUM"))

        # Replicated weights, resident in SBUF for the whole kernel.
        # The y-phase weights are laid out n-half-major so the half
        # needed first (n=0 columns of y) can arrive first.
        waT8_sb = wpool.tile([P, KD, STATE], fp8)
        bT_sb = wpool.tile([P, KD, STATE], bf16)
        cT_sb = wpool.tile([P, 2, KS, 512], bf16)
        dT8_sb = wpool.tile([P, 2, K8, 512], fp8)
        dTbf_sb = wpool.tile([P, 2, KDB, 512], bf16)
        bias_sb = wpool.tile([P, KS], f32)

        x8_tiles = []
        xbf_tiles = []

        def prefetch_xs(c):
            cs = CHUNKS[c]
            lo = KD * STARTS[c]
            t8 = xpool.tile([P, KD, cs], fp8, tag="x8")
            nc.sync.dma_start(t8[:], x8[:, lo : lo + KD * cs])
            x8_tiles.append(t8)
            tb = xpool.tile([P, KD, cs], bf16, tag="xbf")
            nc.sync.dma_start(tb[:], xbf[:, lo : lo + KD * cs])
            xbf_tiles.append(tb)

        # Ramp: the PE consumes x8+xbf at ~300GB/s through the first
        # ab-phases, and the DMA system sustains ~330GB/s aggregate with
        # the active queues sharing it roughly fairly. Chunks 0-1 are
        # small (256) so early PE demand stays under the DMA supply
        # curve, and the ramp set (chunks 0-2 + all weights) is cut into
        # ~128-512KB pieces assigned round-robin across the three queues
        # in PE need order: each queue's FIFO then tracks global need
        # order and no queue burns shared bandwidth on data needed 10us
        # later.
        for c in range(3):
            x8_tiles.append(
                xpool.tile([P, KD, CHUNKS[c]], fp8, tag="x8", name=f"x8t{c}")
            )
            xbf_tiles.append(
                xpool.tile([P, KD, CHUNKS[c]], bf16, tag="xbf", name=f"xbft{c}")
            )
        cs0, cs1, cs2 = CHUNKS[0], CHUNKS[1], CHUNKS[2]
        lo1, lo2 = KD * STARTS[1], KD * STARTS[2]
        YW = 512  # n-half weight width

        pieces = [
            (waT8_sb[:, 0:2, :], waT8[:, 0 : 2 * STATE]),
            (x8_tiles[0][:, 0:4, :], x8[:, 0 : 4 * cs0]),
            (waT8_sb[:, 2:8, :], waT8[:, 2 * STATE : 8 * STATE]),
            (x8_tiles[0][:, 4:8, :], x8[:, 4 * cs0 : 8 * cs0]),
            (bias_sb[:], bias[:]),
            (bT_sb[:, 0:4, :], bT[:, 0 : 4 * STATE]),
            (xbf_tiles[0][:, 0:4, :], xbf[:, 0 : 4 * cs0]),
            (bT_sb[:, 4:8, :], bT[:, 4 * STATE : 8 * STATE]),
            # xbf c0's second half split in two so the critical prefix
            # stays balanced across the three fair-shared queues.
            (xbf_tiles[0][:, 4:6, :], xbf[:, 4 * cs0 : 6 * cs0]),
            (xbf_tiles[0][:, 6:8, :], xbf[:, 6 * cs0 : 8 * cs0]),
            (x8_tiles[1][:], x8[:, lo1 : lo1 + KD * cs1]),
            (xbf_tiles[1][:, 0:4, :], xbf[:, lo1 : lo1 + 4 * cs1]),
            (xbf_tiles[1][:, 4:8, :], xbf[:, lo1 + 4 * cs1 : lo1 + 8 * cs1]),
            (cT_sb[:, 0], cT[:, 0 : KS * YW]),
            (dTbf_sb[:, 0], dTbf[:, 0 : KDB * YW]),
            (dT8_sb[:, 0], dT8[:, 0 : K8 * YW]),
            (x8_tiles[2][:], x8[:, lo2 : lo2 + KD * cs2]),
            (cT_sb[:, 1], cT[:, KS * YW : 2 * KS * YW]),
            (dTbf_sb[:, 1], dTbf[:, KDB * YW : 2 * KDB * YW]),
            (dT8_sb[:, 1], dT8[:, K8 * YW : 2 * K8 * YW]),
            (xbf_tiles[2][:, 0:4, :], xbf[:, lo2 : lo2 + 4 * cs2]),
            (xbf_tiles[2][:, 4:8, :], xbf[:, lo2 + 4 * cs2 : lo2 + 8 * cs2]),
        ]
        rr = [nc.sync, nc.scalar, nc.gpsimd]
        for i, (dst, src) in enumerate(pieces):
            rr[i % 3].dma_start(dst, src)

        h_tiles = {}
        hbf_tiles = {}

        def emit_ab(c):
            cs = CHUNKS[c]
            x8t = x8_tiles[c]
            xbt = xbf_tiles[c]
            # PSUM tiles padded to 512 so a 384-token chunk's s=1 slice
            # doesn't straddle a 2KB bank boundary.
            a_ps = pa.tile([P, KS, 512], f32, tag="a_ps")
            b_ps = pb.tile([P, KS, 512], f32, tag="b_ps")
            a_sb = apool.tile([P, KS, cs], f32, tag="a_sb")
            h_sb = hpool.tile([P, KS, cs], f32, tag="h_sb")
            prev_h = h_tiles.get(c - 1)
            h_bf = hbfpool.tile([P, KS, cs], bf16, tag="h_bf")
            # Both a s-groups back-to-back: a single fp8/bf16 PE mode
            # transition per chunk.
            for s in range(KS):
                for kp in range(KD // 2):
                    nc.tensor.matmul(
                        a_ps[:, s, :cs],
                        waT8_sb[:, 2 * kp : 2 * kp + 2, ts(s, P)],
                        x8t[:, 2 * kp : 2 * kp + 2, :],
                        start=(kp == 0),
                        stop=(kp == KD // 2 - 1),
                        perf_mode=DR,
                    )
            for s in range(KS):
                nc.scalar.activation(
                    a_sb[:, s, :], a_ps[:, s, :cs], AF.Sigmoid,
                    bias=bias_sb[:, s : s + 1],
                )
                for k in range(KD):
                    nc.tensor.matmul(
                        b_ps[:, s, :cs],
                        bT_sb[:, k, ts(s, P)],
                        xbt[:, k, :],
                        start=(k == 0),
                        stop=(k == KD - 1),
                    )
                init = 0.0 if prev_h is None else prev_h[:, s, CHUNKS[c - 1] - 1 : CHUNKS[c - 1]]
                nc.vector.tensor_tensor_scan(
                    h_sb[:, s, :], a_sb[:, s, :], b_ps[:, s, :cs], init,
                    op0=ALU.mult, op1=ALU.add,
                )
                nc.vector.tensor_copy(h_bf[:, s, :], h_sb[:, s, :])
            h_tiles[c] = h_sb
            hbf_tiles[c] = h_bf

        def emit_y(c, last=False):
            tt = CHUNKS[c] // P
            row0 = STARTS[c] // P
            x8t = x8_tiles[c]
            xbt = xbf_tiles[c]
            h_bf = hbf_tiles[c]
            y_sb = ypool.tile([P, tt, D_MODEL], f32, tag="y_sb")

            def mm_c(y_ps, t, n, first):
                for s in range(KS):
                    nc.tensor.matmul(
                        y_ps[:],
                        h_bf[:, s, ts(t, P)],
                        cT_sb[:, n, s, :],
                        start=(first and s == 0),
                        stop=False,
                    )

            def mm_dbf(y_ps, t, n):
                for k in range(KDB):
                    nc.tensor.matmul(
                        y_ps[:],
                        xbt[:, (K8OFF + K8 + k) % KD, ts(t, P)],
                        dTbf_sb[:, n, k, :],
                        start=False,
                        stop=False,
                    )

            def mm_d8(y_ps, t, n):
                for kp in range(K8 // 2):
                    nc.tensor.matmul(
                        y_ps[:],
                        x8t[:, K8OFF + 2 * kp : K8OFF + 2 * kp + 2, ts(t, P)],
                        dT8_sb[:, n, 2 * kp : 2 * kp + 2, :],
                        start=False,
                        stop=(kp == K8 // 2 - 1),
                        perf_mode=DR,
                    )

            # Process one n-half of all four t-blocks at a time: the four
            # bf16 (C + D-bf16) groups first, then the four fp8 DoubleRow
            # runs back to back -- 3 fp8<->bf16 PE mode transitions per
            # chunk (the n=1 DR run lands adjacent to the next chunk's
            # DoubleRow a-matmuls). The four open PSUM tiles exactly fill
            # the 4-buffer py pool.
            for n in range(2):
                tiles = {}
                for t in range(tt):
                    y_ps = py.tile([P, 512], f32)
                    tiles[t] = y_ps
                    mm_c(y_ps, t, n, True)
                    mm_dbf(y_ps, t, n)
                for t in range(tt):
                    mm_d8(tiles[t], t, n)
                for t in range(tt):
                    if last:
                        # Tail trim: pipeline the final copies and stores
                        # at 256-col granularity across engines/queues so
                        # the last HBM write starts ~1.5us earlier.
                        for h in range(2):
                            cl = n * 512 + h * 256
                            if h == 0:
                                nc.vector.tensor_copy(
                                    y_sb[:, t, cl : cl + 256],
                                    tiles[t][:, h * 256 : h * 256 + 256],
                                )
                            else:
                                nc.scalar.copy(
                                    y_sb[:, t, cl : cl + 256],
                                    tiles[t][:, h * 256 : h * 256 + 256],
                                )
                            q = [nc.scalar, nc.gpsimd, nc.sync, nc.scalar][2 * n + h]
                            q.dma_start(
                                y[ts(row0 + t, P), cl : cl + 256],
                                y_sb[:, t, cl : cl + 256],
                            )
                        continue
                    if t % 2 == 0:
                        nc.vector.tensor_copy(y_sb[:, t, ts(n, 512)], tiles[t][:])
                    else:
                        nc.scalar.copy(y_sb[:, t, ts(n, 512)], tiles[t][:])
                    if n == 1:
                        nc.scalar.dma_start(y[ts(row0 + t, P), :], y_sb[:, t, :])

        # Emission order doubles as scheduler priority. y(c) is emitted
        # right after ab(c): the list scheduler still fills y's scan-wait
        # stalls with ab(c+1) matmuls (data-ready), but among ready work
        # it prefers finishing y(c)'s fp8 run over interleaving b(c+1)
        # bf16 matmuls into it (fewer PE mode transitions).
        for c in range(NCH):
            if c >= 1 and c + 2 < NCH:
                prefetch_xs(c + 2)
            emit_ab(c)
            emit_y(c, last=(c == NCH - 1))

    nc.compile()
    return nc


_NC_CACHE = None
LAST_RESULTS = None


def _chunked(xT):
    """[D_MODEL, SEQ] -> [P, KD*SEQ] with chunk-c columns holding the
    k-major [KD, cs] block, so chunk DMAs are contiguous."""
    a = xT.reshape(KD, P, SEQ)
    cols = []
    for c, cs in enumerate(CHUNKS):
        blk = a[:, :, STARTS[c] : STARTS[c] + cs]  # [KD, P, cs]
        cols.append(blk.transpose(1, 0, 2).reshape(P, KD * cs))
    return np.ascontiguousarray(np.concatenate(cols, axis=1))


def _slabbed(w):
    """[K*P, M] -> [P, K*M] k-major per partition."""
    k = w.shape[0] // P
    m = w.shape[1]
    return np.ascontiguousarray(
        w.reshape(k, P, m).transpose(1, 0, 2).reshape(P, k * m)
    )


def _slabbed_nhalf(w):
    """[K*P, 1024] -> [P, 2*K*512]: n-half-major, then k-major."""
    k = w.shape[0] // P
    return np.ascontiguousarray(
        w.reshape(k, P, 2, 512).transpose(1, 2, 0, 3).reshape(P, 2 * k * 512)
    )


def kernel(x, Wa_w, Wa_b, B_w, C_w, D_w):
    global _NC_CACHE, LAST_RESULTS
    if _NC_CACHE is None:
        _NC_CACHE = _build_nc()
    nc = _NC_CACHE

    import ml_dtypes

    F8 = ml_dtypes.float8_e4m3fn
    BF = ml_dtypes.bfloat16
    up = float(2.0**SD)
    dn = float(2.0**-SD)

    x = np.asarray(x, dtype=np.float32)
    waT8 = _slabbed((np.ascontiguousarray(np.asarray(Wa_w, np.float32).T) * up).astype(F8))
    bT = _slabbed(np.ascontiguousarray(np.asarray(B_w, np.float32).T).astype(BF))
    cT = _slabbed_nhalf(np.ascontiguousarray(np.asarray(C_w, np.float32).T).astype(BF))
    dT = np.ascontiguousarray(np.asarray(D_w, np.float32).T)
    dT8 = _slabbed_nhalf((dT[K8OFF * P : (K8OFF + K8) * P] * up).astype(F8))
    dTbf = _slabbed_nhalf(
        np.ascontiguousarray(
            np.concatenate([dT[: K8OFF * P], dT[(K8OFF + K8) * P :]])
        ).astype(BF)
    )
    bias = np.ascontiguousarray(np.asarray(Wa_b, np.float32).reshape(KS, P).T)

    in_maps = []
    for i in range(N_CORES):
        xT = np.ascontiguousarray(x[i].T)
        in_maps.append(
            {
                "x8": _chunked((xT * dn).astype(F8)),
                "xbf": _chunked(xT.astype(BF)),
                "waT8": waT8,
                "bT": bT,
                "cT": cT,
                "dT8": dT8,
                "dTbf": dTbf,
                "bias": bias,
            }
        )

    LAST_RESULTS = run_bass_kernel_spmd(nc, in_maps, core_ids=list(range(N_CORES)))
    return np.stack([r["y"] for r in LAST_RESULTS.results], axis=0)


# revision 19
# speedup vs baseline: 1.2419x; 1.2419x over previous
"""Trainium2 Bass kernel for a diagonal SSM layer.

Reference computation (per batch row b, seq t):
    a_t = sigmoid(Wa @ x_t + bias)        [state=256]
    b_t = B @ x_t                         [state=256]
    h_t = a_t * h_{t-1} + b_t             (linear scan over t)
    y_t = C @ h_t + D @ x_t               [d_model=1024]

Distribution: data-parallel over batch (8 rows -> 8 NeuronCores),
weights replicated. Host pre-transposes and pre-quantizes the streams:
 - a-matmul runs fully in fp8(e4m3) DoubleRow mode (2 k-slabs per
   instruction, 2x the bf16 MAC rate); the sigmoid's flat slope at
   logit ~2.2 makes the quantization error negligible.
 - D-matmul contracts 6 of its 8 K-slabs in fp8 DoubleRow and 2 in
   bf16 -- the error budget (rel 2e-2 vs the f32 reference) allows fp8
   on only part of the dominant D@x term. The slab choice and the
   exponent shift were picked by an exact numpy simulation of the
   quantization error (which matches HW to ~4 digits).
 - b-matmul and C-matmul stay bf16; b's error is amplified ~1.3x by the
   scan so it cannot afford fp8.
 - fp8 operands are exponent-shifted (x*2^-2, weights*2^2) so the
   uniform-distributed weights clear the e4m3 subnormal cutoff.
The scan runs along the SBUF free dimension via the hardware
TensorTensorScan instruction. The PE pays ~190ns per bf16<->fp8 mode
transition, so DoubleRow matmuls are batched into long runs.

All HBM-side tensors are pre-arranged on the host into the exact
[partition, free] order the SBUF tiles use, so every DMA moves long
contiguous runs (2-8KB per partition) instead of 256-512B strided
rows -- the baseline's rearranging DMAs ran at ~73GB/s and starved
the PE for the first 12us.
"""

import sys
import types

sys.path.insert(0, "/opt/trn_rl_repo")


def _ensure_axon_hooks_shim():
    # Some images lack antenv.axon_hooks; concourse imports it
    # unconditionally when BASS_TRACE is set. Provide a no-op shim so
    # tracing degrades gracefully instead of crashing.
    try:
        import antenv.axon_hooks  # noqa: F401
        return
    except ImportError:
        pass
    import antenv

    mod = types.ModuleType("antenv.axon_hooks")
    mod._hook = None

    def get_axon_ntff_profile_hook():
        return mod._hook

    def set_axon_ntff_profile_hook(hook):
        mod._hook = hook

    mod.get_axon_ntff_profile_hook = get_axon_ntff_profile_hook
    mod.set_axon_ntff_profile_hook = set_axon_ntff_profile_hook
    sys.modules["antenv.axon_hooks"] = mod
    antenv.axon_hooks = mod


_ensure_axon_hooks_shim()

from contextlib import ExitStack

import numpy as np

from concourse import bacc, bass, mybir, tile
from concourse.bass_utils import run_bass_kernel_spmd

D_MODEL = 1024
STATE = 256
SEQ = 4096
BATCH = 8
N_CORES = 8
P = 128

KD = D_MODEL // P  # 8 k-slabs over d_model
KS = STATE // P  # 2 slabs over state
K8 = 6  # k-slabs of the D contraction done in fp8 DoubleRow (must be even)
KDB = KD - K8  # bf16 k-slabs for D
K8OFF = 2  # first fp8 k-slab (slabs K8OFF..K8OFF+K8-1 are fp8, rest bf16)
SD = 2  # fp8 exponent shift: x*2^-SD, weights*2^SD
CHUNKS = [256, 256] + [512] * 6 + [384, 128]
STARTS = [sum(CHUNKS[:i]) for i in range(len(CHUNKS))]
NCH = len(CHUNKS)

f32 = mybir.dt.float32
bf16 = mybir.dt.bfloat16
fp8 = mybir.dt.float8e4
ts = bass.ts
AF = mybir.ActivationFunctionType
ALU = mybir.AluOpType
DR = mybir.MatmulPerfMode.DoubleRow


def _build_nc():
    nc = bacc.Bacc("TRN2", target_bir_lowering=False, debug=False)

    # All inputs are host-pre-arranged [128, free] with the free dim in
    # the exact k-major order the SBUF tiles use: column c of row p is
    # contiguous, so each DMA is 128 descriptors of 2-8KB.
    x8 = nc.dram_tensor("x8", [P, KD * SEQ], fp8, kind="ExternalInput").ap()
    xbf = nc.dram_tensor("xbf", [P, KD * SEQ], bf16, kind="ExternalInput").ap()
    waT8 = nc.dram_tensor("waT8", [P, KD * STATE], fp8, kind="ExternalInput").ap()
    bT = nc.dram_tensor("bT", [P, KD * STATE], bf16, kind="ExternalInput").ap()
    cT = nc.dram_tensor("cT", [P, KS * D_MODEL], bf16, kind="ExternalInput").ap()
    dT8 = nc.dram_tensor("dT8", [P, K8 * D_MODEL], fp8, kind="ExternalInput").ap()
    dTbf = nc.dram_tensor("dTbf", [P, KDB * D_MODEL], bf16, kind="ExternalInput").ap()
    bias = nc.dram_tensor("bias", [P, KS], f32, kind="ExternalInput").ap()
    y = nc.dram_tensor("y", [SEQ, D_MODEL], f32, kind="ExternalOutput").ap()

    with tile.TileContext(nc) as tc, ExitStack() as ctx:
        wpool = ctx.enter_context(tc.tile_pool(name="w", bufs=1))
        xpool = ctx.enter_context(tc.tile_pool(name="x", bufs=4))
        apool = ctx.enter_context(tc.tile_pool(name="a", bufs=2))
        hpool = ctx.enter_context(tc.tile_pool(name="h", bufs=2))
        ypool = ctx.enter_context(tc.tile_pool(name="yo", bufs=2))
        hbfpool = ctx.enter_context(tc.tile_pool(name="hbf", bufs=2))
        pa = ctx.enter_context(tc.tile_pool(name="pa", bufs=1, space="PSUM"))
        pb = ctx.enter_context(tc.tile_pool(name="pb", bufs=1, space="PSUM"))
        py = ctx.enter_context(tc.tile_pool(name="py", bufs=4, space="PSUM"))

        # Replicated weights, resident in SBUF for the whole kernel.
        # The y-phase weights are laid out n-half-major so the half
        # needed first (n=0 columns of y) can arrive first.
        waT8_sb = wpool.tile([P, KD, STATE], fp8)
        bT_sb = wpool.tile([P, KD, STATE], bf16)
        cT_sb = wpool.tile([P, 2, KS, 512], bf16)
        dT8_sb = wpool.tile([P, 2, K8, 512], fp8)
        dTbf_sb = wpool.tile([P, 2, KDB, 512], bf16)
        bias_sb = wpool.tile([P, KS], f32)

        x8_tiles = []
        xbf_tiles = []

        def prefetch_xs(c):
            cs = CHUNKS[c]
            lo = KD * STARTS[c]
            t8 = xpool.tile([P, KD, cs], fp8, tag="x8")
            nc.sync.dma_start(t8[:], x8[:, lo : lo + KD * cs])
            x8_tiles.append(t8)
            tb = xpool.tile([P, KD, cs], bf16, tag="xbf")
            nc.sync.dma_start(tb[:], xbf[:, lo : lo + KD * cs])
            xbf_tiles.append(tb)

        # Ramp: the PE consumes x8+xbf at ~300GB/s through the first
        # ab-phases, and the DMA system sustains ~330GB/s aggregate with
        # the active queues sharing it roughly fairly. Chunks 0-1 are
        # small (256) so early PE demand stays under the DMA supply
        # curve, and the ramp set (chunks 0-2 + all weights) is cut into
        # ~128-512KB pieces assigned round-robin across the three queues
        # in PE need order: each queue's FIFO then tracks global need
        # order and no queue burns shared bandwidth on data needed 10us
        # later.
        for c in range(3):
            x8_tiles.append(
                xpool.tile([P, KD, CHUNKS[c]], fp8, tag="x8", name=f"x8t{c}")
            )
            xbf_tiles.append(
                xpool.tile([P, KD, CHUNKS[c]], bf16, tag="xbf", name=f"xbft{c}")
            )
        cs0, cs1, cs2 = CHUNKS[0], CHUNKS[1], CHUNKS[2]
        lo1, lo2 = KD * STARTS[1], KD * STARTS[2]
        YW = 512  # n-half weight width

        pieces = [
            (waT8_sb[:, 0:2, :], waT8[:, 0 : 2 * STATE]),
            (x8_tiles[0][:, 0:4, :], x8[:, 0 : 4 * cs0]),
            (waT8_sb[:, 2:8, :], waT8[:, 2 * STATE : 8 * STATE]),
            (x8_tiles[0][:, 4:8, :], x8[:, 4 * cs0 : 8 * cs0]),
            (bias_sb[:], bias[:]),
            (bT_sb[:, 0:4, :], bT[:, 0 : 4 * STATE]),
            (xbf_tiles[0][:, 0:4, :], xbf[:, 0 : 4 * cs0]),
            (bT_sb[:, 4:8, :], bT[:, 4 * STATE : 8 * STATE]),
            # xbf c0's second half split in two so the critical prefix
            # stays balanced across the three fair-shared queues.
            (xbf_tiles[0][:, 4:6, :], xbf[:, 4 * cs0 : 6 * cs0]),
            (xbf_tiles[0][:, 6:8, :], xbf[:, 6 * cs0 : 8 * cs0]),
            (x8_tiles[1][:], x8[:, lo1 : lo1 + KD * cs1]),
            (xbf_tiles[1][:, 0:4, :], xbf[:, lo1 : lo1 + 4 * cs1]),
            (xbf_tiles[1][:, 4:8, :], xbf[:, lo1 + 4 * cs1 : lo1 + 8 * cs1]),
            (cT_sb[:, 0], cT[:, 0 : KS * YW]),
            (dTbf_sb[:, 0], dTbf[:, 0 : KDB * YW]),
            (dT8_sb[:, 0], dT8[:, 0 : K8 * YW]),
            (x8_tiles[2][:], x8[:, lo2 : lo2 + KD * cs2]),
            (cT_sb[:, 1], cT[:, KS * YW : 2 * KS * YW]),
            (dTbf_sb[:, 1], dTbf[:, KDB * YW : 2 * KDB * YW]),
            (dT8_sb[:, 1], dT8[:, K8 * YW : 2 * K8 * YW]),
            (xbf_tiles[2][:, 0:4, :], xbf[:, lo2 : lo2 + 4 * cs2]),
            (xbf_tiles[2][:, 4:8, :], xbf[:, lo2 + 4 * cs2 : lo2 + 8 * cs2]),
        ]
        rr = [nc.sync, nc.scalar, nc.gpsimd]
        for i, (dst, src) in enumerate(pieces):
            rr[i % 3].dma_start(dst, src)

        h_tiles = {}
        hbf_tiles = {}

        def emit_ab(c):
            cs = CHUNKS[c]
            x8t = x8_tiles[c]
            xbt = xbf_tiles[c]
            # PSUM tiles padded to 512 so a 384-token chunk's s=1 slice
            # doesn't straddle a 2KB bank boundary.
            a_ps = pa.tile([P, KS, 512], f32, tag="a_ps")
            b_ps = pb.tile([P, KS, 512], f32, tag="b_ps")
            a_sb = apool.tile([P, KS, cs], f32, tag="a_sb")
            h_sb = hpool.tile([P, KS, cs], f32, tag="h_sb")
            prev_h = h_tiles.get(c - 1)
            h_bf = hbfpool.tile([P, KS, cs], bf16, tag="h_bf")
            # Both a s-groups back-to-back: a single fp8/bf16 PE mode
            # transition per chunk.
            for s in range(KS):
                for kp in range(KD // 2):
                    nc.tensor.matmul(
                        a_ps[:, s, :cs],
                        waT8_sb[:, 2 * kp : 2 * kp + 2, ts(s, P)],
                        x8t[:, 2 * kp : 2 * kp + 2, :],
                        start=(kp == 0),
                        stop=(kp == KD // 2 - 1),
                        perf_mode=DR,
                    )
            for s in range(KS):
                nc.scalar.activation(
                    a_sb[:, s, :], a_ps[:, s, :cs], AF.Sigmoid,
                    bias=bias_sb[:, s : s + 1],
                )
                for k in range(KD):
                    nc.tensor.matmul(
                        b_ps[:, s, :cs],
                        bT_sb[:, k, ts(s, P)],
                        xbt[:, k, :],
                        start=(k == 0),
                        stop=(k == KD - 1),
                    )
                init = 0.0 if prev_h is None else prev_h[:, s, CHUNKS[c - 1] - 1 : CHUNKS[c - 1]]
                nc.vector.tensor_tensor_scan(
                    h_sb[:, s, :], a_sb[:, s, :], b_ps[:, s, :cs], init,
                    op0=ALU.mult, op1=ALU.add,
                )
                nc.vector.tensor_copy(h_bf[:, s, :], h_sb[:, s, :])
            h_tiles[c] = h_sb
            hbf_tiles[c] = h_bf

        def emit_y(c, last=False):
            tt = CHUNKS[c] // P
            row0 = STARTS[c] // P
            x8t = x8_tiles[c]
            xbt = xbf_tiles[c]
            h_bf = hbf_tiles[c]
            y_sb = ypool.tile([P, tt, D_MODEL], f32, tag="y_sb")

            def mm_c(y_ps, t, n, first):
                for s in range(KS):
                    nc.tensor.matmul(
                        y_ps[:],
                        h_bf[:, s, ts(t, P)],
                        cT_sb[:, n, s, :],
                        start=(first and s == 0),
                        stop=False,
                    )

            def mm_dbf(y_ps, t, n):
                for k in range(KDB):
                    nc.tensor.matmul(
                        y_ps[:],
                        xbt[:, (K8OFF + K8 + k) % KD, ts(t, P)],
                        dTbf_sb[:, n, k, :],
                        start=False,
                        stop=False,
                    )

            def mm_d8(y_ps, t, n):
                for kp in range(K8 // 2):
                    nc.tensor.matmul(
                        y_ps[:],
                        x8t[:, K8OFF + 2 * kp : K8OFF + 2 * kp + 2, ts(t, P)],
                        dT8_sb[:, n, 2 * kp : 2 * kp + 2, :],
                        start=False,
                        stop=(kp == K8 // 2 - 1),
                        perf_mode=DR,
                    )

            # Process one n-half of all four t-blocks at a time: the four
            # bf16 (C + D-bf16) groups first, then the four fp8 DoubleRow
            # runs back to back -- 3 fp8<->bf16 PE mode transitions per
            # chunk (the n=1 DR run lands adjacent to the next chunk's
            # DoubleRow a-matmuls). The four open PSUM tiles exactly fill
            # the 4-buffer py pool.
            for n in range(2):
                tiles = {}
                for t in range(tt):
                    y_ps = py.tile([P, 512], f32)
                    tiles[t] = y_ps
                    mm_c(y_ps, t, n, True)
                    mm_dbf(y_ps, t, n)
                for t in range(tt):
                    mm_d8(tiles[t], t, n)
                for t in range(tt):
                    if last:
                        # Tail trim: pipeline the final copies and stores
                        # at 256-col granularity across engines/queues so
                        # the last HBM write starts ~1.5us earlier.
                        for h in range(2):
                            cl = n * 512 + h * 256
                            if h == 0:
                                nc.vector.tensor_copy(
                                    y_sb[:, t, cl : cl + 256],
                                    tiles[t][:, h * 256 : h * 256 + 256],
                                )
                            else:
                                nc.scalar.copy(
                                    y_sb[:, t, cl : cl + 256],
                                    tiles[t][:, h * 256 : h * 256 + 256],
                                )
                            q = [nc.scalar, nc.gpsimd, nc.sync, nc.scalar][2 * n + h]
                            q.dma_start(
                                y[ts(row0 + t, P), cl : cl + 256],
                                y_sb[:, t, cl : cl + 256],
                            )
                        continue
                    # Split each PSUM->SBUF copy across vector+scalar so
                    # the PSUM tile frees in ~350ns instead of ~690ns --
                    # faster tile recycling keeps the n=1 matmuls from
                    # ping-ponging with next-chunk work (PE mode churn).
                    cl = n * 512
                    nc.vector.tensor_copy(
                        y_sb[:, t, cl : cl + 256], tiles[t][:, 0:256]
                    )
                    nc.scalar.copy(
                        y_sb[:, t, cl + 256 : cl + 512], tiles[t][:, 256:512]
                    )
                    if n == 1:
                        nc.scalar.dma_start(y[ts(row0 + t, P), :], y_sb[:, t, :])

        # Software pipeline: y-phase for chunk c runs while chunk c+1's
        # a/b matmuls fill the PE queue, hiding the sigmoid+scan latency
        # behind matmul work.
        for c in range(NCH):
            if c >= 1 and c + 2 < NCH:
                prefetch_xs(c + 2)
            emit_ab(c)
            if c >= 1:
                emit_y(c - 1)
        emit_y(NCH - 1, last=True)

    nc.compile()
    return nc


_NC_CACHE = None
LAST_RESULTS = None


def _chunked(xT):
    """[D_MODEL, SEQ] -> [P, KD*SEQ] with chunk-c columns holding the
    k-major [KD, cs] block, so chunk DMAs are contiguous."""
    a = xT.reshape(KD, P, SEQ)
    cols = []
    for c, cs in enumerate(CHUNKS):
        blk = a[:, :, STARTS[c] : STARTS[c] + cs]  # [KD, P, cs]
        cols.append(blk.transpose(1, 0, 2).reshape(P, KD * cs))
    return np.ascontiguousarray(np.concatenate(cols, axis=1))


def _slabbed(w):
    """[K*P, M] -> [P, K*M] k-major per partition."""
    k = w.shape[0] // P
    m = w.shape[1]
    return np.ascontiguousarray(
        w.reshape(k, P, m).transpose(1, 0, 2).reshape(P, k * m)
    )


def _slabbed_nhalf(w):
    """[K*P, 1024] -> [P, 2*K*512]: n-half-major, then k-major."""
    k = w.shape[0] // P
    return np.ascontiguousarray(
        w.reshape(k, P, 2, 512).transpose(1, 2, 0, 3).reshape(P, 2 * k * 512)
    )


def kernel(x, Wa_w, Wa_b, B_w, C_w, D_w):
    global _NC_CACHE, LAST_RESULTS
    if _NC_CACHE is None:
        _NC_CACHE = _build_nc()
    nc = _NC_CACHE

    import ml_dtypes

    F8 = ml_dtypes.float8_e4m3fn
    BF = ml_dtypes.bfloat16
    up = float(2.0**SD)
    dn = float(2.0**-SD)

    x = np.asarray(x, dtype=np.float32)
    waT8 = _slabbed((np.ascontiguousarray(np.asarray(Wa_w, np.float32).T) * up).astype(F8))
    bT = _slabbed(np.ascontiguousarray(np.asarray(B_w, np.float32).T).astype(BF))
    cT = _slabbed_nhalf(np.ascontiguousarray(np.asarray(C_w, np.float32).T).astype(BF))
    dT = np.ascontiguousarray(np.asarray(D_w, np.float32).T)
    dT8 = _slabbed_nhalf((dT[K8OFF * P : (K8OFF + K8) * P] * up).astype(F8))
    dTbf = _slabbed_nhalf(
        np.ascontiguousarray(
            np.concatenate([dT[: K8OFF * P], dT[(K8OFF + K8) * P :]])
        ).astype(BF)
    )
    bias = np.ascontiguousarray(np.asarray(Wa_b, np.float32).reshape(KS, P).T)

    in_maps = []
    for i in range(N_CORES):
        xT = np.ascontiguousarray(x[i].T)
        in_maps.append(
            {
                "x8": _chunked((xT * dn).astype(F8)),
                "xbf": _chunked(xT.astype(BF)),
                "waT8": waT8,
                "bT": bT,
                "cT": cT,
                "dT8": dT8,
                "dTbf": dTbf,
                "bias": bias,
            }
        )

    LAST_RESULTS = run_bass_kernel_spmd(nc, in_maps, core_ids=list(range(N_CORES)))
    return np.stack([r["y"] for r in LAST_RESULTS.results], axis=0)


# revision 21
# speedup vs baseline: 1.2865x; 1.0359x over previous
"""Trainium2 Bass kernel for a diagonal SSM layer.

Reference computation (per batch row b, seq t):
    a_t = sigmoid(Wa @ x_t + bias)        [state=256]
    b_t = B @ x_t                         [state=256]
    h_t = a_t * h_{t-1} + b_t             (linear scan over t)
    y_t = C @ h_t + D @ x_t               [d_model=1024]

Distribution: data-parallel over batch (8 rows -> 8 NeuronCores),
weights replicated. Host pre-transposes and pre-quantizes the streams:
 - a-matmul runs fully in fp8(e4m3) DoubleRow mode (2 k-slabs per
   instruction, 2x the bf16 MAC rate); the sigmoid's flat slope at
   logit ~2.2 makes the quantization error negligible.
 - D-matmul contracts 6 of its 8 K-slabs in fp8 DoubleRow and 2 in
   bf16 -- the error budget (rel 2e-2 vs the f32 reference) allows fp8
   on only part of the dominant D@x term. The slab choice and the
   exponent shift were picked by an exact numpy simulation of the
   quantization error (which matches HW to ~4 digits).
 - b-matmul and C-matmul stay bf16; b's error is amplified ~1.3x by the
   scan so it cannot afford fp8.
 - fp8 operands are exponent-shifted (x*2^-2, weights*2^2) so the
   uniform-distributed weights clear the e4m3 subnormal cutoff.
The scan runs along the SBUF free dimension via the hardware
TensorTensorScan instruction. The PE pays ~190ns per bf16<->fp8 mode
transition, so DoubleRow matmuls are batched into long runs.

All HBM-side tensors are pre-arranged on the host into the exact
[partition, free] order the SBUF tiles use, so every DMA moves long
contiguous runs (2-8KB per partition) instead of 256-512B strided
rows -- the baseline's rearranging DMAs ran at ~73GB/s and starved
the PE for the first 12us.
"""

import sys
import types

sys.path.insert(0, "/opt/trn_rl_repo")


def _ensure_axon_hooks_shim():
    # Some images lack antenv.axon_hooks; concourse imports it
    # unconditionally when BASS_TRACE is set. Provide a no-op shim so
    # tracing degrades gracefully instead of crashing.
    try:
        import antenv.axon_hooks  # noqa: F401
        return
    except ImportError:
        pass
    import antenv

    mod = types.ModuleType("antenv.axon_hooks")
    mod._hook = None

    def get_axon_ntff_profile_hook():
        return mod._hook

    def set_axon_ntff_profile_hook(hook):
        mod._hook = hook

    mod.get_axon_ntff_profile_hook = get_axon_ntff_profile_hook
    mod.set_axon_ntff_profile_hook = set_axon_ntff_profile_hook
    sys.modules["antenv.axon_hooks"] = mod
    antenv.axon_hooks = mod


_ensure_axon_hooks_shim()

from contextlib import ExitStack

import numpy as np

from concourse import bacc, bass, mybir, tile
from concourse.bass_utils import run_bass_kernel_spmd

D_MODEL = 1024
STATE = 256
SEQ = 4096
BATCH = 8
N_CORES = 8
P = 128

KD = D_MODEL // P  # 8 k-slabs over d_model
KS = STATE // P  # 2 slabs over state
K8 = 6  # k-slabs of the D contraction done in fp8 DoubleRow (must be even)
KDB = KD - K8  # bf16 k-slabs for D
K8OFF = 2  # first fp8 k-slab (slabs K8OFF..K8OFF+K8-1 are fp8, rest bf16)
SD = 2  # fp8 exponent shift: x*2^-SD, weights*2^SD
CHUNKS = [256, 384] + [512] * 6 + [256, 128]
STARTS = [sum(CHUNKS[:i]) for i in range(len(CHUNKS))]
NCH = len(CHUNKS)

f32 = mybir.dt.float32
bf16 = mybir.dt.bfloat16
fp8 = mybir.dt.float8e4
ts = bass.ts
AF = mybir.ActivationFunctionType
ALU = mybir.AluOpType
DR = mybir.MatmulPerfMode.DoubleRow


def _build_nc():
    nc = bacc.Bacc("TRN2", target_bir_lowering=False, debug=False)

    # All inputs are host-pre-arranged [128, free] with the free dim in
    # the exact k-major order the SBUF tiles use: column c of row p is
    # contiguous, so each DMA is 128 descriptors of 2-8KB.
    x8 = nc.dram_tensor("x8", [P, KD * SEQ], fp8, kind="ExternalInput").ap()
    xbf = nc.dram_tensor("xbf", [P, KD * SEQ], bf16, kind="ExternalInput").ap()
    waT8 = nc.dram_tensor("waT8", [P, KD * STATE], fp8, kind="ExternalInput").ap()
    bT = nc.dram_tensor("bT", [P, KD * STATE], bf16, kind="ExternalInput").ap()
    cT = nc.dram_tensor("cT", [P, KS * D_MODEL], bf16, kind="ExternalInput").ap()
    dT8 = nc.dram_tensor("dT8", [P, K8 * D_MODEL], fp8, kind="ExternalInput").ap()
    dTbf = nc.dram_tensor("dTbf", [P, KDB * D_MODEL], bf16, kind="ExternalInput").ap()
    bias = nc.dram_tensor("bias", [P, KS], f32, kind="ExternalInput").ap()
    y = nc.dram_tensor("y", [SEQ, D_MODEL], f32, kind="ExternalOutput").ap()

    with tile.TileContext(nc) as tc, ExitStack() as ctx:
        wpool = ctx.enter_context(tc.tile_pool(name="w", bufs=1))
        xpool = ctx.enter_context(tc.tile_pool(name="x", bufs=4))
        apool = ctx.enter_context(tc.tile_pool(name="a", bufs=2))
        hpool = ctx.enter_context(tc.tile_pool(name="h", bufs=2))
        ypool = ctx.enter_context(tc.tile_pool(name="yo", bufs=2))
        hbfpool = ctx.enter_context(tc.tile_pool(name="hbf", bufs=2))
        pa = ctx.enter_context(tc.tile_pool(name="pa", bufs=1, space="PSUM"))
        pb = ctx.enter_context(tc.tile_pool(name="pb", bufs=1, space="PSUM"))
        py = ctx.enter_context(tc.tile_pool(name="py", bufs=4, space="PSUM"))

        # Replicated weights, resident in SBUF for the whole kernel.
        # The y-phase weights are laid out n-half-major so the half
        # needed first (n=0 columns of y) can arrive first.
        waT8_sb = wpool.tile([P, KD, STATE], fp8)
        bT_sb = wpool.tile([P, KD, STATE], bf16)
        cT_sb = wpool.tile([P, 2, KS, 512], bf16)
        dT8_sb = wpool.tile([P, 2, K8, 512], fp8)
        dTbf_sb = wpool.tile([P, 2, KDB, 512], bf16)
        bias_sb = wpool.tile([P, KS], f32)

        x8_tiles = []
        xbf_tiles = []

        def prefetch_xs(c):
            cs = CHUNKS[c]
            lo = KD * STARTS[c]
            t8 = xpool.tile([P, KD, cs], fp8, tag="x8")
            nc.sync.dma_start(t8[:], x8[:, lo : lo + KD * cs])
            x8_tiles.append(t8)
            tb = xpool.tile([P, KD, cs], bf16, tag="xbf")
            nc.sync.dma_start(tb[:], xbf[:, lo : lo + KD * cs])
            xbf_tiles.append(tb)

        # Ramp: the PE consumes x8+xbf at ~300GB/s through the first
        # ab-phases, and the DMA system sustains ~330GB/s aggregate with
        # the active queues sharing it roughly fairly. Chunks 0-1 are
        # small (256) so early PE demand stays under the DMA supply
        # curve, and the ramp set (chunks 0-2 + all weights) is cut into
        # ~128-512KB pieces assigned round-robin across the three queues
        # in PE need order: each queue's FIFO then tracks global need
        # order and no queue burns shared bandwidth on data needed 10us
        # later.
        for c in range(3):
            x8_tiles.append(
                xpool.tile([P, KD, CHUNKS[c]], fp8, tag="x8", name=f"x8t{c}")
            )
            xbf_tiles.append(
                xpool.tile([P, KD, CHUNKS[c]], bf16, tag="xbf", name=f"xbft{c}")
            )
        cs0, cs1, cs2 = CHUNKS[0], CHUNKS[1], CHUNKS[2]
        lo1, lo2 = KD * STARTS[1], KD * STARTS[2]
        YW = 512  # n-half weight width

        pieces = [
            (waT8_sb[:, 0:2, :], waT8[:, 0 : 2 * STATE]),
            (x8_tiles[0][:, 0:4, :], x8[:, 0 : 4 * cs0]),
            (waT8_sb[:, 2:8, :], waT8[:, 2 * STATE : 8 * STATE]),
            (x8_tiles[0][:, 4:8, :], x8[:, 4 * cs0 : 8 * cs0]),
            (bias_sb[:], bias[:]),
            (bT_sb[:, 0:4, :], bT[:, 0 : 4 * STATE]),
            (xbf_tiles[0][:, 0:4, :], xbf[:, 0 : 4 * cs0]),
            (bT_sb[:, 4:8, :], bT[:, 4 * STATE : 8 * STATE]),
            # xbf c0's second half split in two so the critical prefix
            # stays balanced across the three fair-shared queues.
            (xbf_tiles[0][:, 4:6, :], xbf[:, 4 * cs0 : 6 * cs0]),
            (xbf_tiles[0][:, 6:8, :], xbf[:, 6 * cs0 : 8 * cs0]),
            (x8_tiles[1][:], x8[:, lo1 : lo1 + KD * cs1]),
            (xbf_tiles[1][:, 0:4, :], xbf[:, lo1 : lo1 + 4 * cs1]),
            (xbf_tiles[1][:, 4:8, :], xbf[:, lo1 + 4 * cs1 : lo1 + 8 * cs1]),
            (cT_sb[:, 0], cT[:, 0 : KS * YW]),
            (dTbf_sb[:, 0], dTbf[:, 0 : KDB * YW]),
            (dT8_sb[:, 0], dT8[:, 0 : K8 * YW]),
            (x8_tiles[2][:], x8[:, lo2 : lo2 + KD * cs2]),
            (cT_sb[:, 1], cT[:, KS * YW : 2 * KS * YW]),
            (dTbf_sb[:, 1], dTbf[:, KDB * YW : 2 * KDB * YW]),
            (dT8_sb[:, 1], dT8[:, K8 * YW : 2 * K8 * YW]),
            (xbf_tiles[2][:, 0:4, :], xbf[:, lo2 : lo2 + 4 * cs2]),
            (xbf_tiles[2][:, 4:8, :], xbf[:, lo2 + 4 * cs2 : lo2 + 8 * cs2]),
        ]
        rr = [nc.sync, nc.scalar, nc.gpsimd]
        for i, (dst, src) in enumerate(pieces):
            rr[i % 3].dma_start(dst, src)

        h_tiles = {}
        hbf_tiles = {}

        def emit_ab(c):
            cs = CHUNKS[c]
            x8t = x8_tiles[c]
            xbt = xbf_tiles[c]
            # PSUM tiles padded to 512 so a 384-token chunk's s=1 slice
            # doesn't straddle a 2KB bank boundary.
            a_ps = pa.tile([P, KS, 512], f32, tag="a_ps")
            b_ps = pb.tile([P, KS, 512], f32, tag="b_ps")
            a_sb = apool.tile([P, KS, cs], f32, tag="a_sb")
            h_sb = hpool.tile([P, KS, cs], f32, tag="h_sb")
            prev_h = h_tiles.get(c - 1)
            h_bf = hbfpool.tile([P, KS, cs], bf16, tag="h_bf")
            # Both a s-groups back-to-back: a single fp8/bf16 PE mode
            # transition per chunk.
            for s in range(KS):
                for kp in range(KD // 2):
                    nc.tensor.matmul(
                        a_ps[:, s, :cs],
                        waT8_sb[:, 2 * kp : 2 * kp + 2, ts(s, P)],
                        x8t[:, 2 * kp : 2 * kp + 2, :],
                        start=(kp == 0),
                        stop=(kp == KD // 2 - 1),
                        perf_mode=DR,
                    )
            for s in range(KS):
                nc.scalar.activation(
                    a_sb[:, s, :], a_ps[:, s, :cs], AF.Sigmoid,
                    bias=bias_sb[:, s : s + 1],
                )
                for k in range(KD):
                    nc.tensor.matmul(
                        b_ps[:, s, :cs],
                        bT_sb[:, k, ts(s, P)],
                        xbt[:, k, :],
                        start=(k == 0),
                        stop=(k == KD - 1),
                    )
                init = 0.0 if prev_h is None else prev_h[:, s, CHUNKS[c - 1] - 1 : CHUNKS[c - 1]]
                nc.vector.tensor_tensor_scan(
                    h_sb[:, s, :], a_sb[:, s, :], b_ps[:, s, :cs], init,
                    op0=ALU.mult, op1=ALU.add,
                )
                nc.vector.tensor_copy(h_bf[:, s, :], h_sb[:, s, :])
            h_tiles[c] = h_sb
            hbf_tiles[c] = h_bf

        def emit_y(c, last=False):
            tt = CHUNKS[c] // P
            row0 = STARTS[c] // P
            x8t = x8_tiles[c]
            xbt = xbf_tiles[c]
            h_bf = hbf_tiles[c]
            y_sb = ypool.tile([P, tt, D_MODEL], f32, tag="y_sb")

            def mm_c(y_ps, t, n, first):
                for s in range(KS):
                    nc.tensor.matmul(
                        y_ps[:],
                        h_bf[:, s, ts(t, P)],
                        cT_sb[:, n, s, :],
                        start=(first and s == 0),
                        stop=False,
                    )

            def mm_dbf(y_ps, t, n):
                for k in range(KDB):
                    nc.tensor.matmul(
                        y_ps[:],
                        xbt[:, (K8OFF + K8 + k) % KD, ts(t, P)],
                        dTbf_sb[:, n, k, :],
                        start=False,
                        stop=False,
                    )

            def mm_d8(y_ps, t, n):
                for kp in range(K8 // 2):
                    nc.tensor.matmul(
                        y_ps[:],
                        x8t[:, K8OFF + 2 * kp : K8OFF + 2 * kp + 2, ts(t, P)],
                        dT8_sb[:, n, 2 * kp : 2 * kp + 2, :],
                        start=False,
                        stop=(kp == K8 // 2 - 1),
                        perf_mode=DR,
                    )

            # Process one n-half of all four t-blocks at a time: the four
            # bf16 (C + D-bf16) groups first, then the four fp8 DoubleRow
            # runs back to back -- 3 fp8<->bf16 PE mode transitions per
            # chunk (the n=1 DR run lands adjacent to the next chunk's
            # DoubleRow a-matmuls). The four open PSUM tiles exactly fill
            # the 4-buffer py pool.
            for n in range(2):
                tiles = {}
                for t in range(tt):
                    y_ps = py.tile([P, 512], f32)
                    tiles[t] = y_ps
                    mm_c(y_ps, t, n, True)
                    mm_dbf(y_ps, t, n)
                for t in range(tt):
                    mm_d8(tiles[t], t, n)
                for t in range(tt):
                    if last:
                        # Tail trim: pipeline the final copies and stores
                        # at 256-col granularity across engines/queues so
                        # the last HBM write starts ~1.5us earlier.
                        for h in range(2):
                            cl = n * 512 + h * 256
                            if h == 0:
                                nc.vector.tensor_copy(
                                    y_sb[:, t, cl : cl + 256],
                                    tiles[t][:, h * 256 : h * 256 + 256],
                                )
                            else:
                                nc.scalar.copy(
                                    y_sb[:, t, cl : cl + 256],
                                    tiles[t][:, h * 256 : h * 256 + 256],
                                )
                            q = [nc.scalar, nc.gpsimd, nc.sync, nc.scalar][2 * n + h]
                            q.dma_start(
                                y[ts(row0 + t, P), cl : cl + 256],
                                y_sb[:, t, cl : cl + 256],
                            )
                        continue
                    if t % 2 == 0:
                        nc.vector.tensor_copy(y_sb[:, t, ts(n, 512)], tiles[t][:])
                    else:
                        nc.scalar.copy(y_sb[:, t, ts(n, 512)], tiles[t][:])
                    if n == 1:
                        nc.scalar.dma_start(y[ts(row0 + t, P), :], y_sb[:, t, :])

        # Software pipeline: y-phase for chunk c runs while chunk c+1's
        # a/b matmuls fill the PE queue, hiding the sigmoid+scan latency
        # behind matmul work.
        for c in range(NCH):
            if c >= 1 and c + 2 < NCH:
                prefetch_xs(c + 2)
            emit_ab(c)
            if c >= 1:
                emit_y(c - 1)
        emit_y(NCH - 1, last=True)

    nc.compile()
    return nc


_NC_CACHE = None
LAST_RESULTS = None


def _chunked(xT):
    """[D_MODEL, SEQ] -> [P, KD*SEQ] with chunk-c columns holding the
    k-major [KD, cs] block, so chunk DMAs are contiguous."""
    a = xT.reshape(KD, P, SEQ)
    cols = []
    for c, cs in enumerate(CHUNKS):
        blk = a[:, :, STARTS[c] : STARTS[c] + cs]  # [KD, P, cs]
        cols.append(blk.transpose(1, 0, 2).reshape(P, KD * cs))
    return np.ascontiguousarray(np.concatenate(cols, axis=1))


def _slabbed(w):
    """[K*P, M] -> [P, K*M] k-major per partition."""
    k = w.shape[0] // P
    m = w.shape[1]
    return np.ascontiguousarray(
        w.reshape(k, P, m).transpose(1, 0, 2).reshape(P, k * m)
    )


def _slabbed_nhalf(w):
    """[K*P, 1024] -> [P, 2*K*512]: n-half-major, then k-major."""
    k = w.shape[0] // P
    return np.ascontiguousarray(
        w.reshape(k, P, 2, 512).transpose(1, 2, 0, 3).reshape(P, 2 * k * 512)
    )


def kernel(x, Wa_w, Wa_b, B_w, C_w, D_w):
    global _NC_CACHE, LAST_RESULTS
    if _NC_CACHE is None:
        _NC_CACHE = _build_nc()
    nc = _NC_CACHE

    import ml_dtypes

    F8 = ml_dtypes.float8_e4m3fn
    BF = ml_dtypes.bfloat16
    up = float(2.0**SD)
    dn = float(2.0**-SD)

    x = np.asarray(x, dtype=np.float32)
    waT8 = _slabbed((np.ascontiguousarray(np.asarray(Wa_w, np.float32).T) * up).astype(F8))
    bT = _slabbed(np.ascontiguousarray(np.asarray(B_w, np.float32).T).astype(BF))
    cT = _slabbed_nhalf(np.ascontiguousarray(np.asarray(C_w, np.float32).T).astype(BF))
    dT = np.ascontiguousarray(np.asarray(D_w, np.float32).T)
    dT8 = _slabbed_nhalf((dT[K8OFF * P : (K8OFF + K8) * P] * up).astype(F8))
    dTbf = _slabbed_nhalf(
        np.ascontiguousarray(
            np.concatenate([dT[: K8OFF * P], dT[(K8OFF + K8) * P :]])
        ).astype(BF)
    )
    bias = np.ascontiguousarray(np.asarray(Wa_b, np.float32).reshape(KS, P).T)

    in_maps = []
    for i in range(N_CORES):
        xT = np.ascontiguousarray(x[i].T)
        in_maps.append(
            {
                "x8": _chunked((xT * dn).astype(F8)),
                "xbf": _chunked(xT.astype(BF)),
                "waT8": waT8,
                "bT": bT,
                "cT": cT,
                "dT8": dT8,
                "dTbf": dTbf,
                "bias": bias,
            }
        )

    LAST_RESULTS = run_bass_kernel_spmd(nc, in_maps, core_ids=list(range(N_CORES)))
    return np.stack([r["y"] for r in LAST_RESULTS.results], axis=0)
